# revision 27
# baseline (speedup 1.0000x reference)
"""BatchHardTripletLoss (with faithful source bug) on 8 Trainium2 NeuronCores.

Reference semantics (N=8192, D=128, C=10 classes, margin=1.0):
    d(i,j)   = max(x2_i + x2_j - 2 e_i.e_j, 0)
    d_pos[i] = max_{j: same class} d(i,j)                  (includes self)
    S[i,k]   = sum_{j: class k} d(i,j);  k* = argmax_k S[i,k]
    j*       = (k*)-th negative of i in (class, index) order
    loss     = mean relu(d_pos - d(i,j*) + 1)

Key structure exploited (validated against the reference, ~1e-5 rel):
  * Only the diagonal of d clamps at 0 (off-diagonal distances are ~256 for
    this regime), and the diagonal is exactly 0, so S has the closed form
        S[i,k] = cnt_k*x2_i + C_k - 2 e_i.E_k
    with E_k / C_k / cnt_k per-class sums of embeddings / x2 / counts.
  * k* < 10 <= class sizes, so j* is among the first 10 members of class 0
    (anchors with label != 0) or of class 1 (anchors with label == 0).
  * d_pos only needs distances within the anchor's own class block.
  So the N x N distance matrix is never materialized: per anchor we need its
  class block (~820 cols), 10 S columns, and 10 candidate columns.

Device layout: rows and columns are class-sorted; every class block is padded
to a uniform width B (pad = duplicate of the block's first member, which never
affects a max and whose loss row is squashed via the x2a1 -inf trick). One
NEFF with static shapes serves all 8 cores; per-core variation is data-only.
Each core gets Q = 10*B/128/8 anchor tiles: one whole "main" block plus a
slice of one leftover block, so its column window is exactly 2 blocks.

Per anchor tile t (class block known statically):
  * window: one bf16 matmul per 512-col chunk (lhsT = -2*e_anchor, shared
    weights, back-to-back) into a [128, B] PSUM tile; then ONE custom-DVE
    ADD_MAX_REDUCE (registered below: out = psum + x2_j, accum = row max)
    gives mall[:, t] = max_j(x2_j - 2 e_i.e_j) in a single DVE pass.
  * S + candidates: augmented matmul pair K=128 (bf16) + K=2 (fp32)
      lhsT=[-2 e_i; x2_i; 1], rhs=[E_k; cnt_k; C_k] and [e_c; 1; x2_c]
    giving [S[i,k] | d(i,cand)] in a [128, 20] PSUM tile. The bf16-rounded
    x2_i is reused in x2a1 so the x2_i term cancels in d_pos - d_neg.
Post-loop, batched across tiles: candidate select via (S == rowmax) masked
dot products (scalar_tensor_tensor), then
loss = max(mall + (x2_i + margin | -inf if pad) - d_neg, 0), summed per
partition; host adds the 8 per-core [128] partials.

Host does only O(N*D) input marshalling (sort/pad/stats); all O(N*B*D) work
plus the mining runs on the NeuronCores.
"""

import numpy as np
from contextlib import ExitStack

import ml_dtypes
import concourse.bass as bass
import concourse.tile as tile
from concourse import bacc, mybir
from concourse import dve_ops
from concourse.dve_spec import Spec, Src0, Src1, C0, maxx, lower, _has_src1
from concourse.dve_uop import DveOpSpec
from concourse.bass_utils import run_bass_kernel_spmd

N_CORES = 8
C = 10
MARGIN = 1.0
P = 128
F32 = mybir.dt.float32
BF16 = mybir.dt.bfloat16
AX = mybir.AxisListType.X
ALU = mybir.AluOpType
NEG_INF = -3.0e38
PAD_NEG = -1.0e30

# stash of the last BassKernelResults (read by test.py for profiling)
last_results = None
_trace_opts: dict = {}


def _ref_add_max_reduce(in0, in1, c0, c1, c2):
    b = (np.asarray(in0, np.float32) + np.asarray(in1, np.float32))
    if isinstance(c0, np.ndarray):
        seed = np.asarray(c0, np.float32).reshape(-1, 1)
    else:
        seed = np.full((b.shape[0], 1), float(c0), np.float32)
    acc = np.maximum(seed, b.reshape(b.shape[0], -1).max(axis=-1, keepdims=True))
    return b.astype(np.float32), acc.astype(np.float32)


def _register_add_max_reduce():
    """Custom DVE op: out = in0 + in1; accum_out = max(s0, rowmax(out)).

    Fuses the x2_j broadcast add into the hardest-positive max so the PSUM
    distance tile is consumed in a single DVE pass (the native
    TENSOR_TENSOR_REDUCE ISA op hard-faults on this runtime, and the K=2
    aux-matmul alternative doubles the TensorE column stream)."""
    name = "ADD_MAX_REDUCE_BHTL"
    for op in dve_ops.OPS:
        if op.name == name:
            return op
    spec = Spec(body=Src0 + Src1, accum=maxx, accum_init=C0,
                reference=_ref_add_max_reduce)
    row = dve_ops._CUSTOM_DVE_ROW_BASE + len(dve_ops.OPS)
    assert row < 0x20
    dve_ops._SUB_OPCODE_FOR_NAME[name] = row
    shas = {}
    for ver in ("v3", "v4"):
        try:
            u = lower(spec, ver=ver)
            shas[ver] = DveOpSpec(name=name, opcode=row, uops=u,
                                  rd1_en=_has_src1(spec)).sha(ver)
        except Exception:
            pass
    assert shas, "ADD_MAX_REDUCE_BHTL failed to lower for any DVE version"
    op = dve_ops.DveOp(name, spec, subdim=False, uops_sha=shas)
    dve_ops.OPS.append(op)
    dve_ops.CUSTOM_DVE_SPECS[name] = spec
    return op


ADD_MAX_REDUCE = _register_add_max_reduce()


def _build_program(B: int, Q: int, Wr: int):
    """One SPMD program; all per-core variation is in the input tensors.

    B: padded class-block width (multiple of 512), Q: anchor tiles per core,
    Wr: number of window columns actually read (global max class count —
    columns beyond it are padding in every block, so the max skips them).
    """
    NCH = B // 512  # 512-col chunks per block
    nc = bacc.Bacc("TRN2", target_bir_lowering=False, debug=False,
                   num_devices=N_CORES)

    a_d = nc.dram_tensor("a", [P, Q * P], BF16, kind="ExternalInput").ap()
    a2_d = nc.dram_tensor("a2", [4, Q * P], BF16, kind="ExternalInput").ap()
    w_d = nc.dram_tensor("w", [P, 2 * Wr], BF16, kind="ExternalInput").ap()
    x2j_d = nc.dram_tensor("x2j", [P, 2 * Wr], BF16, kind="ExternalInput").ap()
    sc_d = nc.dram_tensor("sc", [P, Q * 20], BF16, kind="ExternalInput").ap()
    sc2_d = nc.dram_tensor("sc2", [4, Q * 20], BF16, kind="ExternalInput").ap()
    xm_d = nc.dram_tensor("x2a1", [P, Q], F32, kind="ExternalInput").ap()
    out_d = nc.dram_tensor("out", [1, 1], F32, kind="ExternalOutput").ap()

    with tile.TileContext(nc) as tc, ExitStack() as ctx:
        const = ctx.enter_context(tc.tile_pool(name="const", bufs=1))
        psum = ctx.enter_context(tc.tile_pool(name="psum", bufs=3, space="PSUM"))
        psc = ctx.enter_context(tc.tile_pool(name="psc", bufs=1, space="PSUM"))
        scratch = ctx.enter_context(tc.tile_pool(name="scratch", bufs=2))

        # spread input DMAs across 3 queues; tile 0's deps land first, each
        # on a different queue (per-queue transfers serialize at ~50-100GB/s)
        wpw = [min(512, Wr - 512 * (i % NCH)) for i in range(2 * NCH)]
        wp = [const.tile([P, wpw[i]], BF16, name=f"wp{i}", tag=f"wp{i}")
              for i in range(2 * NCH)]
        woff = [(i // NCH) * Wr + (i % NCH) * 512 for i in range(2 * NCH)]
        a0_sb = const.tile([P, 2 * P], BF16)         # anchors, tiles 0-1
        a1_sb = const.tile([P, (Q - 2) * P], BF16)   # anchors, tiles 2..Q-1

        # memsets first: they run on GpSimd, which later also generates SWDGE
        # DMA descriptors (a multi-us drain) — emitting them late stalls PE
        ones_sb = const.tile([P, 1], F32)
        nc.gpsimd.memset(ones_sb[:], 1.0)
        # dummy 1x1 matmul: absorbs the PE sequencer's ~2us first-instruction
        # overhead while the input DMAs are still in flight
        psd = psc.tile([1, 1], F32, tag="pout", name="psd")
        nc.tensor.matmul(psd[:], ones_sb[:], ones_sb[:], start=True, stop=True)

        x2jp = []
        for b in range(2):
            t_ = const.tile([P, Wr], BF16, tag=f"x2jp{b}", name=f"x2jp{b}")
            x2jp.append(t_)
        # wp0's transfer split across two queues (single matmul still reads
        # the whole tile); halves land in parallel ~2us earlier
        nc.sync.dma_start(wp[0][:, 0:256], w_d[:, 0:256])
        nc.gpsimd.dma_start(wp[0][:, 256:512], w_d[:, 256:512])
        nc.gpsimd.dma_start(a0_sb[:], a_d[:, 0:2 * P])
        nc.scalar.dma_start(x2jp[0][:], x2j_d[:, 0:Wr])
        nc.scalar.dma_start(wp[1][:], w_d[:, woff[1]:woff[1] + wpw[1]])
        nc.sync.dma_start(a1_sb[:], a_d[:, 2 * P:Q * P])
        for i in range(2, 2 * NCH):
            nc.sync.dma_start(wp[i][:], w_d[:, woff[i]:woff[i] + wpw[i]])
        sc_sb = const.tile([P, Q * 20], BF16)
        nc.scalar.dma_start(sc_sb[:], sc_d[:])
        sc2_sb = const.tile([4, Q * 20], BF16)
        nc.scalar.dma_start(sc2_sb[:], sc2_d[:])
        a2_sb = const.tile([4, Q * P], BF16)
        nc.scalar.dma_start(a2_sb[:], a2_d[:])
        nc.gpsimd.dma_start(x2jp[1][:], x2j_d[:, Wr:2 * Wr])
        xm_sb = const.tile([P, Q], F32)
        nc.gpsimd.dma_start(xm_sb[:], xm_d[:])

        mall = const.tile([P, Q], F32)         # max_j(x2_j - 2 e_i.e_j)
        sv_all = const.tile([P, Q * 20], F32)  # per-tile [S | d_cand]

        TB = B // P  # tiles in the main block

        def win_lhs(t):
            if t < 2:
                return a0_sb[:, t * P:(t + 1) * P]
            return a1_sb[:, (t - 2) * P:(t - 1) * P]

        ps_tiles = {}

        def emit_window_mms(t):
            blk = 0 if t < TB else 1
            ps = psum.tile([P, B], F32, tag="ps", name=f"ps{t}")
            ps_tiles[t] = ps
            for h in range(NCH):
                n = wpw[NCH * blk + h]
                nc.tensor.matmul(ps[:, 512 * h:512 * h + n], win_lhs(t),
                                 wp[NCH * blk + h][:],
                                 start=True, stop=True)

        for t in range(Q):
            blk = 0 if t < TB else 1
            emit_window_mms(t)
            ps = ps_tiles.pop(t)
            lhs = win_lhs(t)
            lhs2 = a2_sb[:, t * P:(t + 1) * P]

            dsc = scratch.tile([P, B], F32)
            nc.vector._custom_dve(ADD_MAX_REDUCE, out=dsc[:, 0:Wr],
                                  in0=ps[:, 0:Wr], in1=x2jp[blk][:],
                                  s0=NEG_INF, accum_out=mall[:, t:t + 1])

            pv = psc.tile([P, 20], F32)
            scol = slice(t * 20, (t + 1) * 20)
            nc.tensor.matmul(pv[:], lhs, sc_sb[:, scol], start=True, stop=False)
            nc.tensor.matmul(pv[:], lhs2, sc2_sb[:, scol], start=False, stop=True)
            nc.scalar.copy(sv_all[:, scol], pv[:])

        # ---- batched mining epilogue ----
        smax = const.tile([P, Q], F32)
        sv3 = sv_all[:].rearrange("p (q s) -> p q s", s=20)
        nc.vector.reduce_max(smax[:], sv3[:, :, 0:10], axis=AX)
        dneg = const.tile([P, Q], F32)
        for t in range(Q):
            scr = scratch.tile([P, 10], F32, tag="scr")
            nc.vector.scalar_tensor_tensor(
                scr[:], sv_all[:, t * 20:t * 20 + 10], smax[:, t:t + 1],
                sv_all[:, t * 20 + 10:t * 20 + 20],
                op0=ALU.is_equal, op1=ALU.mult, accum_out=dneg[:, t:t + 1])

        t1 = const.tile([P, Q], F32)
        nc.vector.tensor_add(t1[:], mall[:], xm_sb[:])  # + x2_i + margin | -inf
        t2 = const.tile([P, Q], F32)
        nc.vector.tensor_sub(t2[:], t1[:], dneg[:])
        t3 = const.tile([P, Q], F32)
        nc.vector.tensor_scalar(t3[:], t2[:], MARGIN, 0.0,
                                op0=ALU.add, op1=ALU.max)  # relu(x + margin)
        lsum = const.tile([P, 1], F32)
        nc.vector.reduce_sum(lsum[:], t3[:], axis=AX)
        # partition-sum via a 1-column matmul so the output DMA is a single
        # 4-byte transfer (a [128,1] DMA decomposes into 128 descriptors and
        # its completion lags ~12us)
        pout = psc.tile([1, 1], F32, tag="pout")
        nc.tensor.matmul(pout[:], lsum[:], ones_sb[:], start=True, stop=True)
        res_sb = const.tile([1, 1], F32)
        nc.scalar.copy(res_sb[:], pout[:])
        nc.sync.dma_start(out_d[:], res_sb[:])

    nc.compile()
    return nc


_prog_cache: dict = {}


def kernel(embeddings: np.ndarray, labels: np.ndarray) -> np.ndarray:
    global last_results
    e = np.ascontiguousarray(np.asarray(embeddings), dtype=np.float32)
    lab = np.asarray(labels).astype(np.int64)
    N, D = e.shape
    assert D == P and N % N_CORES == 0

    # ---- host-side marshalling: class-sort, pad, per-class stats ----
    order = np.argsort(lab * N + np.arange(N))
    e = e[order]
    lab_s = lab[order]
    cnt = np.bincount(lab_s, minlength=C)
    assert len(cnt) == C and cnt[0] >= 10 and cnt[1] >= 10, cnt
    offs = np.zeros(C + 1, dtype=np.int64)
    offs[1:] = np.cumsum(cnt)

    # block width: multiple of 512 with C*B/128 tiles splitting evenly
    # across 8 cores -> B in {1024, 1536, ...}
    B = 1024
    while cnt.max() > B or (C * (B // P)) % N_CORES != 0:
        B += 512
    TB = B // P
    Q = C * TB // N_CORES
    L = Q - TB  # leftover tiles per core

    x2 = np.einsum("nd,nd->n", e, e).astype(np.float32)
    NP_ = C * B
    ep = np.empty((NP_, D), np.float32)
    x2p = np.empty(NP_, np.float32)
    validp = np.zeros(NP_, np.float32)
    for k in range(C):
        m = int(cnt[k])
        blk = e[offs[k]:offs[k + 1]]
        ep[k * B:k * B + m] = blk
        ep[k * B + m:(k + 1) * B] = blk[0]
        x2p[k * B:k * B + m] = x2[offs[k]:offs[k + 1]]
        x2p[k * B + m:(k + 1) * B] = x2[offs[k]]
        validp[k * B:k * B + m] = 1.0
    # bf16-rounded x2_i, shared by the fp32 aux matmul and x2a1 so the x2_i
    # term cancels exactly in d_pos - d_neg
    x2p_bf32 = x2p.astype(ml_dtypes.bfloat16).astype(np.float32)

    E = np.stack([e[offs[k]:offs[k + 1]].sum(axis=0) for k in range(C)],
                 axis=1).astype(np.float32)          # [D, C]
    Ck = np.array([x2[offs[k]:offs[k + 1]].sum() for k in range(C)],
                  dtype=np.float32)                  # [C]
    candA = e[0:10]                                  # class-0 members
    candB = e[offs[1]:offs[1] + 10]                  # class-1 members
    x2A, x2B = x2[0:10], x2[offs[1]:offs[1] + 10]

    Wr = int(cnt.max())
    key = (B, Q, Wr)
    if key not in _prog_cache:
        _prog_cache[key] = _build_program(B, Q, Wr)
    nc = _prog_cache[key]

    in_maps = []
    for c in range(N_CORES):
        mb = c                        # main block
        eb = N_CORES + (c * L) // TB  # leftover block index
        et = (c * L) % TB             # first leftover tile within it
        rows = np.concatenate([
            np.arange(mb * B, (mb + 1) * B),
            np.arange(eb * B + et * P, eb * B + (et + L) * P),
        ])
        tile_cls = [mb] * TB + [eb] * L
        wcols = np.concatenate([np.arange(mb * B, mb * B + Wr),
                                np.arange(eb * B, eb * B + Wr)])

        anchT = ep[rows].T                          # [D, Q*128]
        a = (-2.0 * anchT).astype(ml_dtypes.bfloat16)
        # aux lhsT rows [x2_i; 1; x2_i; 1] pair with hi/lo-split rhs rows so
        # every aux product is bf16-exact (bf16 alone cannot hold cnt_k / C_k)
        a2 = np.stack([x2p_bf32[rows], np.ones(Q * P, np.float32),
                       x2p_bf32[rows], np.ones(Q * P, np.float32)])
        w = ep[wcols].T.astype(ml_dtypes.bfloat16)
        x2j = np.broadcast_to(
            x2p[wcols][None, :].astype(ml_dtypes.bfloat16),
            (P, 2 * Wr)).copy()
        sc = np.empty((D, Q * 20), np.float32)
        sc2 = np.empty((4, Q * 20), np.float32)
        cnt_f = cnt.astype(np.float32)
        cnt_hi = (cnt // 128 * 128).astype(np.float32)
        cnt_lo = cnt_f - cnt_hi
        Ck_hi = Ck.astype(ml_dtypes.bfloat16).astype(np.float32)
        Ck_lo = Ck - Ck_hi
        x2A_hi = x2A.astype(ml_dtypes.bfloat16).astype(np.float32)
        x2B_hi = x2B.astype(ml_dtypes.bfloat16).astype(np.float32)
        for t in range(Q):
            c0 = tile_cls[t] == 0
            cand = candB if c0 else candA
            x2c_hi = x2B_hi if c0 else x2A_hi
            x2c_lo = (x2B - x2B_hi) if c0 else (x2A - x2A_hi)
            sc[:, t * 20:t * 20 + 10] = E
            sc[:, t * 20 + 10:t * 20 + 20] = cand.T
            sc2[0, t * 20:t * 20 + 10] = cnt_hi
            sc2[1, t * 20:t * 20 + 10] = Ck_hi
            sc2[2, t * 20:t * 20 + 10] = cnt_lo
            sc2[3, t * 20:t * 20 + 10] = Ck_lo
            sc2[0, t * 20 + 10:t * 20 + 20] = 1.0
            sc2[1, t * 20 + 10:t * 20 + 20] = x2c_hi
            sc2[2, t * 20 + 10:t * 20 + 20] = 0.0
            sc2[3, t * 20 + 10:t * 20 + 20] = x2c_lo
        vmask = validp[rows].reshape(Q, P).T
        x2a1 = np.where(vmask > 0.5,
                        x2p_bf32[rows].reshape(Q, P).T,
                        PAD_NEG).astype(np.float32).copy()

        in_maps.append({"a": a, "a2": a2.astype(ml_dtypes.bfloat16),
                        "w": w, "x2j": x2j,
                        "sc": sc.astype(ml_dtypes.bfloat16),
                        "sc2": sc2.astype(ml_dtypes.bfloat16),
                        "x2a1": x2a1})

    res = run_bass_kernel_spmd(nc, in_maps, list(range(N_CORES)), **_trace_opts)
    last_results = res
    total = np.float64(0.0)
    for c in range(N_CORES):
        total += res.results[c]["out"].astype(np.float64).sum()
    return np.asarray(total / N, dtype=np.float32)
